# revision 2
# baseline (speedup 1.0000x reference)
"""Trainium2 Bass kernel for nn_Adapter (Polytropon/LoRA adapter layer).

Math (reference):
    probs = normalize(sigmoid(module_logits[tasks]))           # [bs, n_skills]
    A_r[b] = sum_t probs[b,t] * A[0,t]                         # [din, r]
    B_r[b] = sum_t probs[b,t] * B[0,t]                         # [r, dout]
    out[b] = x[b] @ W.T + bias + (x[b] @ A_r[b]) @ B_r[b] / r

Strategy:
  * The tiny routing/combination math (probs, A_r, B_r) is folded on the host
    into one per-example effective weight  W_eff[b] = W.T + A_r[b] @ B_r[b]/r
    ([din, dout], ~1 GFLOP on host) so the device runs a single dense matmul
    per example:  out[b] = x[b] @ W_eff[b] + bias.
  * Data parallel over batch: bs == 8 == n_cores, one example per NeuronCore,
    no collectives.
  * Per core we compute out^T = W_eff^T-free layout trick: with xT = x[b].T
    staged on host, the TensorEngine computes psum[o_tile, s_tile] =
    sum_k W_eff[k, o_tile]^T-as-stationary @ xT[k, s_tile], i.e. out^T tiles,
    all DMAs fully contiguous.  Host transposes the [dout, sq] result back.
  * Bias is fused into the PSUM->SBUF eviction on the Scalar engine
    (Identity activation with per-partition bias).
"""

import os
import numpy as np
from contextlib import ExitStack

import concourse.tile as tile
from concourse import bacc, mybir
from concourse.bass_utils import run_bass_kernel_spmd

# Problem shapes (hardcoded per spec)
BS = 8
SQ = 2048
DIN = 2048
DOUT = 2048
RANK = 16
N_CORES = 8

P = 128                 # SBUF/PSUM partitions
KT = DIN // P           # 16 k-tiles (contraction)
OT = DOUT // P          # 16 o-tiles (output rows of out^T)
ST_N = 512              # moving free dim per matmul (1 fp32 PSUM bank)
ST = SQ // ST_N         # 4 s-tiles

# matmul dtype: "bf16" | "f32" | "f32r"  (f32r = fp32 storage, fast PE path)
MM_MODE = os.environ.get("BASS_MM_MODE", "bf16")

_DT_MAP = {
    "bf16": mybir.dt.bfloat16,
    "f32": mybir.dt.float32,
    "f32r": mybir.dt.float32r,
}


def _build_nc(mode: str):
    dt_mm = _DT_MAP[mode]
    nc = bacc.Bacc("TRN2", debug=False)
    xt_d = nc.declare_dram_parameter("xt", [DIN, SQ], dt_mm, isOutput=False)
    w_d = nc.declare_dram_parameter("w", [DIN, DOUT], dt_mm, isOutput=False)
    b_d = nc.declare_dram_parameter("bias", [P, OT], mybir.dt.float32, isOutput=False)
    out_d = nc.declare_dram_parameter("out", [DOUT, SQ], mybir.dt.float32, isOutput=True)

    # contraction row i = kt*128 + p  (same mapping for xT and W rows)
    xt_ap = xt_d[:].rearrange("(kt p) s -> p kt s", p=P)   # [128, KT, SQ]
    w_ap = w_d[:].rearrange("(kt p) o -> p kt o", p=P)     # [128, KT, DOUT]

    with tile.TileContext(nc) as tc, ExitStack() as ctx:
        xpool = ctx.enter_context(tc.tile_pool(name="xp", bufs=KT))
        wpool = ctx.enter_context(tc.tile_pool(name="wp", bufs=2))
        opool = ctx.enter_context(tc.tile_pool(name="op", bufs=2))
        cpool = ctx.enter_context(tc.tile_pool(name="cp", bufs=1))
        pspool = ctx.enter_context(tc.tile_pool(name="ps", bufs=8, space="PSUM"))

        bias_sb = cpool.tile([P, OT], mybir.dt.float32)
        nc.sync.dma_start(out=bias_sb[:], in_=b_d[:])

        # x^T fully resident in SBUF, loaded once
        x_tiles = []
        for kt in range(KT):
            x_sb = xpool.tile([P, SQ], dt_mm, tag="xts")
            nc.sync.dma_start(out=x_sb[:], in_=xt_ap[:, kt, :])
            x_tiles.append(x_sb)

        for ot in range(OT):
            # W_eff columns for this o-tile: [128, KT, 128], one DMA
            w_sb = wpool.tile([P, KT, P], dt_mm, tag="wt")
            nc.sync.dma_start(out=w_sb[:], in_=w_ap[:, :, ot * P:(ot + 1) * P])

            psums = [
                pspool.tile([P, ST_N], mybir.dt.float32, tag="psum",
                            name=f"psum_{ot}_{st}")
                for st in range(ST)
            ]
            for kt in range(KT):
                lhsT = w_sb[:, kt, :]
                for st in range(ST):
                    nc.tensor.matmul(
                        psums[st][:],
                        lhsT=lhsT,
                        rhs=x_tiles[kt][:, st * ST_N:(st + 1) * ST_N],
                        start=(kt == 0),
                        stop=(kt == KT - 1),
                    )

            out_sb = opool.tile([P, SQ], mybir.dt.float32, tag="ot")
            for st in range(ST):
                nc.scalar.activation(
                    out_sb[:, st * ST_N:(st + 1) * ST_N],
                    psums[st][:],
                    mybir.ActivationFunctionType.Identity,
                    bias=bias_sb[:, ot:ot + 1],
                )
            nc.sync.dma_start(out=out_d[ot * P:(ot + 1) * P, :], in_=out_sb[:])

    nc.finalize()
    return nc


_NC_CACHE = {}


def _get_nc(mode: str):
    if mode not in _NC_CACHE:
        _NC_CACHE[mode] = _build_nc(mode)
    return _NC_CACHE[mode]


def _host_prepare(x, W, b, module_logits, A, B, tasks, mode: str):
    """Fold routing + LoRA into per-example W_eff; build per-core input maps."""
    x = np.asarray(x, dtype=np.float32)
    W = np.asarray(W, dtype=np.float32)
    b = np.asarray(b, dtype=np.float32)
    module_logits = np.asarray(module_logits, dtype=np.float32)
    A = np.asarray(A, dtype=np.float32)
    B = np.asarray(B, dtype=np.float32)
    tasks = np.asarray(tasks)

    bs = x.shape[0]
    n_splits, n_skills, _, r = A.shape
    assert n_splits == 1

    logits = module_logits[tasks]                      # [bs, n_skills]
    probs = 1.0 / (1.0 + np.exp(-logits.astype(np.float64)))
    probs = probs / (probs.sum(-1, keepdims=True) + 1e-12)
    probs = probs.astype(np.float32)

    A0 = A[0]                                          # [n_skills, din, r]
    B0 = B[0]                                          # [n_skills, r, dout]
    A_r = np.einsum("bt,tir->bir", probs, A0)          # [bs, din, r]
    B_r = np.einsum("bt,tro->bro", probs, B0)          # [bs, r, dout]

    # W_eff[b] = W.T + A_r[b] @ B_r[b] / r   -> [bs, din, dout]
    Weff = np.matmul(A_r, B_r)
    Weff *= 1.0 / r
    Weff += W.T[None, :, :]

    if mode == "bf16":
        import ml_dtypes
        np_dt = ml_dtypes.bfloat16
    else:
        np_dt = np.float32

    b_resh = np.ascontiguousarray(b.reshape(OT, P).T)  # [128, OT]

    in_maps = []
    for c in range(N_CORES):
        in_maps.append({
            "xt": np.ascontiguousarray(x[c].T).astype(np_dt),
            "w": np.ascontiguousarray(Weff[c]).astype(np_dt),
            "bias": b_resh,
        })
    return in_maps


def _run(inputs: dict, mode: str = MM_MODE, trace: bool = False):
    nc = _get_nc(mode)
    in_maps = _host_prepare(**inputs, mode=mode)
    res = run_bass_kernel_spmd(
        nc, in_maps, core_ids=list(range(N_CORES)), trace=trace,
    )
    out = np.empty((BS, SQ, DOUT), dtype=np.float32)
    for c in range(N_CORES):
        out[c] = res.results[c]["out"].T
    return out, res


def kernel(**inputs) -> np.ndarray:
    out, _ = _run(inputs)
    return out


# revision 17
# speedup vs baseline: 270.4289x; 270.4289x over previous
"""Trainium2 Bass kernel for nn_Adapter (Polytropon/LoRA adapter layer).

Math (reference):
    probs = normalize(sigmoid(module_logits[tasks]))           # [bs, n_skills]
    A_r[b] = sum_t probs[b,t] * A[0,t]                         # [din, r]
    B_r[b] = sum_t probs[b,t] * B[0,t]                         # [r, dout]
    out[b] = x[b] @ W.T + bias + (x[b] @ A_r[b]) @ B_r[b] / r

Strategy:
  * The tiny routing/combination math (probs, A_r, B_r) is folded on the host
    into one per-example effective weight  W_eff[b] = W.T + A_r[b] @ B_r[b]/r
    ([din, dout], ~1 GFLOP on host) so the device runs a single dense matmul
    per example:  out[b] = x[b] @ W_eff[b] + bias.
  * Data parallel over batch: bs == 8 == n_cores, one example per NeuronCore,
    no collectives.
  * Per core we compute out^T = W_eff^T-free layout trick: with xT = x[b].T
    staged on host, the TensorEngine computes psum[o_tile, s_tile] =
    sum_k W_eff[k, o_tile]^T-as-stationary @ xT[k, s_tile], i.e. out^T tiles,
    all DMAs fully contiguous.  Host transposes the [dout, sq] result back.
  * Bias is fused into the PSUM->SBUF eviction on the Scalar engine
    (Identity activation with per-partition bias).
"""

import os
import numpy as np
from contextlib import ExitStack

import concourse.tile as tile
from concourse import bacc, mybir
from concourse.bass_utils import run_bass_kernel_spmd

# Problem shapes (hardcoded per spec)
BS = 8
SQ = 2048
DIN = 2048
DOUT = 2048
RANK = 16
N_CORES = 8

P = 128                 # SBUF/PSUM partitions
KT = DIN // P           # 16 k-tiles (contraction)
OT = DOUT // P          # 16 o-tiles (output rows of out^T)
ST_N = 512              # moving free dim per matmul (1 fp32 PSUM bank)
ST = SQ // ST_N         # 4 s-tiles

# matmul dtype: "bf16" | "f32" | "f32r"  (f32r = fp32 storage, fast PE path)
MM_MODE = os.environ.get("BASS_MM_MODE", "bf16")

_DT_MAP = {
    "bf16": mybir.dt.bfloat16,
    "f32": mybir.dt.float32,
    "f32r": mybir.dt.float32r,
}


def _build_nc(mode: str, repeat: int = 1, st_n: int = ST_N,
              variant: str = "stream", st_outer: bool = False,
              bands: int = 1, skip_ldw: bool = False):
    """Build the SPMD graph.

    repeat>1 wraps the compute in a hardware For_i loop -- used only for
    benchmarking (device time scales with repeat while host/dispatch overhead
    stays fixed).
    variant: "stream" (x resident, W streamed per o-tile) or "resident"
             (x AND W fully resident; loads hoisted out of the repeat loop --
             isolates PE+epilogue steady-state rate).
    st_outer: flip matmul loop nest to change stationary-weight reuse runs.
    bands: 1 = full 128-row matmuls; 2 = two 64-row tile_position bands with
           separate PSUM banks (hides LDWEIGHTS under the other band's MMs),
           combined during eviction.
    """
    import contextlib

    ST_N_ = st_n
    ST_ = SQ // ST_N_
    dt_mm = _DT_MAP[mode]
    if skip_ldw:
        # non-self-loading matmuls fail walrus codegen for fp32/fp32r
        assert mode == "bf16" and bands == 1 and not st_outer
    nc = bacc.Bacc("TRN2", debug=False)
    xt_d = nc.declare_dram_parameter("xt", [DIN, SQ], dt_mm, isOutput=False)
    w_d = nc.declare_dram_parameter("w", [DIN, DOUT], dt_mm, isOutput=False)
    b_d = nc.declare_dram_parameter("bias", [P, OT], mybir.dt.float32, isOutput=False)
    out_d = nc.declare_dram_parameter("out", [DOUT, SQ], mybir.dt.float32, isOutput=True)

    # contraction row i = kt*128 + p  (same mapping for xT and W rows)
    xt_ap = xt_d[:].rearrange("(kt p) s -> p kt s", p=P)   # [128, KT, SQ]
    w_ap = w_d[:].rearrange("(kt p) o -> p kt o", p=P)     # [128, KT, DOUT]

    with tile.TileContext(nc) as tc, ExitStack() as ctx:
        xpool = ctx.enter_context(tc.tile_pool(name="xp", bufs=KT))
        wpool = ctx.enter_context(tc.tile_pool(name="wp", bufs=2))
        opool = ctx.enter_context(tc.tile_pool(name="op", bufs=2))
        cpool = ctx.enter_context(tc.tile_pool(name="cp", bufs=1))
        pspool = ctx.enter_context(tc.tile_pool(name="ps", bufs=8, space="PSUM"))

        def load_consts():
            bias_sb = cpool.tile([P, OT], mybir.dt.float32, name="bias_sb")
            nc.sync.dma_start(out=bias_sb[:], in_=b_d[:])
            x_tiles = []
            for kt in range(KT):
                x_sb = xpool.tile([P, SQ], dt_mm, tag="xts", name=f"x_sb{kt}")
                nc.sync.dma_start(out=x_sb[:], in_=xt_ap[:, kt, :])
                x_tiles.append(x_sb)
            return bias_sb, x_tiles

        def body(bias_sb, x_tiles, w_all):
            for ot in range(OT):
                if w_all is None:
                    # W_eff columns for this o-tile: [128, KT, 128], one DMA
                    # on the ACT HWDGE ring (x loads own the SP ring).
                    w_sb = wpool.tile([P, KT, P], dt_mm, tag="wt",
                                      name=f"w_sb{ot}")
                    nc.scalar.dma_start(
                        out=w_sb[:], in_=w_ap[:, :, ot * P:(ot + 1) * P])
                    lhsT_of = lambda kt, w_sb=w_sb: w_sb[:, kt, :]
                else:
                    lhsT_of = lambda kt, ot=ot: w_all[:, kt, ot * P:(ot + 1) * P]

                if bands == 1:
                    psums = [
                        pspool.tile([P, ST_N_], mybir.dt.float32, tag="psum",
                                    name=f"psum_{ot}_{st}")
                        for st in range(ST_)
                    ]
                    if st_outer:
                        for st in range(ST_):
                            for kt in range(KT):
                                nc.tensor.matmul(
                                    psums[st][:],
                                    lhsT=lhsT_of(kt),
                                    rhs=x_tiles[kt][:, st * ST_N_:(st + 1) * ST_N_],
                                    start=(kt == 0),
                                    stop=(kt == KT - 1),
                                )
                    else:
                        for kt in range(KT):
                            lhsT = lhsT_of(kt)
                            for st in range(ST_):
                                mm = nc.tensor.matmul(
                                    psums[st][:],
                                    lhsT=lhsT,
                                    rhs=x_tiles[kt][:, st * ST_N_:(st + 1) * ST_N_],
                                    start=(kt == 0),
                                    stop=(kt == KT - 1),
                                )
                                if skip_ldw and st > 0:
                                    # weights already resident in the PE array
                                    # from this kt-run's first MM; verified
                                    # post-scheduling by _verify_ldw_stream
                                    mm.ins.ldweights = False
                else:
                    assert bands == 2 and ST_ == 4
                    HB = P // 2  # 64-row band
                    psums = [
                        [pspool.tile([P, ST_N_], mybir.dt.float32, tag="psum",
                                     name=f"psum_{ot}_{st}_{b}")
                         for b in range(2)]
                        for st in range(ST_)
                    ]
                    for kt in range(KT):
                        lhsT = lhsT_of(kt)
                        for st in range(ST_):
                            for b in range(2):
                                nc.tensor.matmul(
                                    psums[st][b][:],
                                    lhsT=lhsT[b * HB:(b + 1) * HB, :],
                                    rhs=x_tiles[kt][b * HB:(b + 1) * HB,
                                                    st * ST_N_:(st + 1) * ST_N_],
                                    start=(kt == 0),
                                    stop=(kt == KT - 1),
                                    tile_position=(b * HB, 0),
                                )

                out_sb = opool.tile([P, SQ], mybir.dt.float32, tag="ot",
                                    name=f"out_sb{ot}")
                for st in range(ST_):
                    if bands == 1:
                        nc.scalar.activation(
                            out_sb[:, st * ST_N_:(st + 1) * ST_N_],
                            psums[st][:],
                            mybir.ActivationFunctionType.Identity,
                            bias=bias_sb[:, ot:ot + 1],
                        )
                    else:
                        nc.scalar.activation(
                            out_sb[:, st * ST_N_:(st + 1) * ST_N_],
                            psums[st][0][:],
                            mybir.ActivationFunctionType.Identity,
                            bias=bias_sb[:, ot:ot + 1],
                        )
                        nc.vector.tensor_add(
                            out_sb[:, st * ST_N_:(st + 1) * ST_N_],
                            out_sb[:, st * ST_N_:(st + 1) * ST_N_],
                            psums[st][1][:],
                        )
                nc.sync.dma_start(out=out_d[ot * P:(ot + 1) * P, :], in_=out_sb[:])

        loop_cm = tc.For_i(0, repeat) if repeat > 1 else contextlib.nullcontext()
        if variant == "resident":
            bias_sb, x_tiles = load_consts()
            w_all = cpool.tile([P, KT, DOUT], dt_mm, name="w_all")
            nc.sync.dma_start(out=w_all[:], in_=w_ap)
            with loop_cm:
                body(bias_sb, x_tiles, w_all)
        else:
            with loop_cm:
                bias_sb, x_tiles = load_consts()
                body(bias_sb, x_tiles, None)

    nc.finalize()
    if skip_ldw:
        _verify_ldw_stream(nc)
    return nc


def _verify_ldw_stream(nc):
    """Static proof that every ldweights=False matmul is preceded in the
    final PE instruction stream by a matmul that loaded the same weights AP
    (no foreign self-loading matmul in between)."""
    from concourse import mybir as mb

    def ap_key(ap):
        return str(ap)

    n_skip = 0
    for bb in nc.main_func.blocks:
        loaded = None  # weights AP currently in the PE array (None = unknown)
        for inst in bb.instructions:
            if isinstance(inst, mb.InstMatmult):
                wkey = ap_key(inst.ins[1])
                if inst.ldweights is False:
                    assert loaded == wkey, (
                        f"ldweights=False matmul {inst.name} but PE array "
                        f"holds {loaded!r}, needs {wkey!r}")
                    n_skip += 1
                else:
                    loaded = wkey
            elif isinstance(inst, mb.InstLdweights):
                loaded = ap_key(inst.ins[0])
    assert n_skip > 0, "skip_ldw built but no ldweights=False matmuls found"


_NC_CACHE = {}


def _get_nc(mode: str):
    if mode not in _NC_CACHE:
        _NC_CACHE[mode] = _build_nc(mode)
    return _NC_CACHE[mode]


def _host_prepare(x, W, b, module_logits, A, B, tasks, mode: str):
    """Fold routing + LoRA into per-example W_eff; build per-core input maps."""
    x = np.asarray(x, dtype=np.float32)
    W = np.asarray(W, dtype=np.float32)
    b = np.asarray(b, dtype=np.float32)
    module_logits = np.asarray(module_logits, dtype=np.float32)
    A = np.asarray(A, dtype=np.float32)
    B = np.asarray(B, dtype=np.float32)
    tasks = np.asarray(tasks)

    bs = x.shape[0]
    n_splits, n_skills, _, r = A.shape
    assert n_splits == 1

    logits = module_logits[tasks]                      # [bs, n_skills]
    probs = 1.0 / (1.0 + np.exp(-logits.astype(np.float64)))
    probs = probs / (probs.sum(-1, keepdims=True) + 1e-12)
    probs = probs.astype(np.float32)

    A0 = A[0]                                          # [n_skills, din, r]
    B0 = B[0]                                          # [n_skills, r, dout]
    A_r = np.einsum("bt,tir->bir", probs, A0)          # [bs, din, r]
    B_r = np.einsum("bt,tro->bro", probs, B0)          # [bs, r, dout]

    # W_eff[b] = W.T + A_r[b] @ B_r[b] / r   -> [bs, din, dout]
    Weff = np.matmul(A_r, B_r)
    Weff *= 1.0 / r
    Weff += W.T[None, :, :]

    if mode == "bf16":
        import ml_dtypes
        np_dt = ml_dtypes.bfloat16
    else:
        np_dt = np.float32

    b_resh = np.ascontiguousarray(b.reshape(OT, P).T)  # [128, OT]

    in_maps = []
    for c in range(N_CORES):
        in_maps.append({
            "xt": np.ascontiguousarray(x[c].T).astype(np_dt),
            "w": np.ascontiguousarray(Weff[c]).astype(np_dt),
            "bias": b_resh,
        })
    return in_maps


def _run(inputs: dict, mode: str = MM_MODE, trace: bool = False):
    nc = _get_nc(mode)
    in_maps = _host_prepare(**inputs, mode=mode)
    last_err = None
    for attempt in range(3):
        try:
            res = run_bass_kernel_spmd(
                nc, in_maps, core_ids=list(range(N_CORES)), trace=trace,
            )
            break
        except Exception as e:  # transient device wedges (NRT_EXEC_UNIT_...)
            last_err = e
            if attempt == 2:
                raise
            import time as _time
            _time.sleep(5)
    out = np.empty((BS, SQ, DOUT), dtype=np.float32)
    for c in range(N_CORES):
        out[c] = res.results[c]["out"].T
    return out, res


def kernel(**inputs) -> np.ndarray:
    out, _ = _run(inputs)
    return out
